# revision 1
# baseline (speedup 1.0000x reference)
"""Trainium2 Bass kernel for the batched constant-velocity Kalman filter.

Key structure exploited:
  * The Kalman covariance recursion is data-independent, so the per-step
    gains and output stats (sx, sy, rho) are batch-wide scalars computed on
    host. rho is exactly 0 (x/y decoupled), and sx == sy.
  * Only the state mean is per-trajectory work: a short scalar-gain
    recursion over 9 observation steps, then a closed-form linear
    extrapolation for the prediction steps.
  * Output is [T_est+len_pred, B, 5] = ~102 MB -> the kernel is dominated
    by the output DMA writes; compute (DVE/ACT elementwise) hides under it.

Sharding: pure data parallel over batch, B=131072 -> 16384 per core x 8.

Per-core layout: batch shard as [128 partitions x 128 lanes], b = p*128 + j.
Input tile per step: [128, 256] (j,c interleaved, contiguous 1 KB/partition).
Output tile per step: [128, 640] (j,ch interleaved with ch in {px,py,sx,sy,rho},
contiguous 2.5 KB/partition), written by one contiguous 320 KB DMA per step.
"""

import numpy as np

DT = 0.1
EPS = 0.01
N_CORES = 8
B_FULL = 131072
B_SHARD = B_FULL // N_CORES  # 16384
T_OBS = 10
P = 128                       # SBUF partitions
J = B_SHARD // P              # 128 lanes per partition


def _scalar_kalman(sigma_a, sigma_obs, sigma_init, n_est, len_pred):
    """Host-side data-independent 2x2 covariance recursion (float64).

    Returns per-step gains a[n_est], b[n_est] and stats sx[n_est+len_pred].
    """
    sa2 = float(sigma_a) ** 2
    r = float(sigma_obs) ** 2
    F = np.array([[1.0, DT], [0.0, 1.0]])
    G = np.array([DT * DT / 2.0, DT])
    Q = sa2 * np.outer(G, G)
    Pc = (float(sigma_init) ** 2) * np.eye(2)
    a_l, b_l, sx_l = [], [], []
    for _ in range(n_est):
        Pc = F @ Pc @ F.T + Q
        S = Pc[0, 0] + r
        a = Pc[0, 0] / S
        b = Pc[1, 0] / S
        IKH = np.array([[1.0 - a, 0.0], [-b, 1.0]])
        Pc = IKH @ Pc @ IKH.T + r * np.outer([a, b], [a, b])
        a_l.append(a)
        b_l.append(b)
        sx_l.append(np.sqrt(max(Pc[0, 0], EPS * EPS)))
    for _ in range(len_pred):
        Pc = F @ Pc @ F.T + Q
        sx_l.append(np.sqrt(max(Pc[0, 0], EPS * EPS)))
    return np.array(a_l), np.array(b_l), np.array(sx_l)


_CACHE = {}


def _build(sigma_a, sigma_obs, sigma_init, len_pred):
    import concourse.bacc as bacc
    import concourse.mybir as mybir
    import concourse.tile as tile

    AF = mybir.ActivationFunctionType
    OP = mybir.AluOpType
    F32 = mybir.dt.float32

    n_est = T_OBS - 1
    n_out = n_est + len_pred
    a_g, b_g, sx_g = _scalar_kalman(sigma_a, sigma_obs, sigma_init, n_est, len_pred)
    # round through f32 so immediates match what f32 math would use
    a_g = a_g.astype(np.float32)
    b_g = b_g.astype(np.float32)
    sx_g = sx_g.astype(np.float32)
    dt = np.float32(DT)

    nc = bacc.Bacc(
        "TRN2",
        target_bir_lowering=False,
        debug=False,
        enable_asserts=False,
        num_devices=N_CORES,
    )
    x = nc.dram_tensor("x", [T_OBS, B_SHARD, 2], F32, kind="ExternalInput")
    y = nc.dram_tensor("y", [n_out, B_SHARD, 5], F32, kind="ExternalOutput")
    x_ap = x.ap()
    y_ap = y.ap()

    with tile.TileContext(nc) as tc:
        with (
            tc.tile_pool(name="zp", bufs=1) as zp,
            tc.tile_pool(name="sp", bufs=1) as sp,
            tc.tile_pool(name="op", bufs=6) as op_,
        ):
            # load all observation tiles; [128, 256] = (j, c) interleaved
            zs = []
            for s in range(T_OBS):
                z = zp.tile([P, 2 * J], F32, name=f"z{s}")
                nc.sync.dma_start(
                    z, x_ap[s].rearrange("(p j) c -> p (j c)", p=P)
                )
                zs.append(z)
            z3 = [z.rearrange("p (j c) -> p j c", c=2) for z in zs]

            dummy = sp.tile([P, J], F32, name="dummy")
            nc.vector.memset(dummy, 0.0)

            # state tiles, persistent across steps
            px = sp.tile([P, J], F32, name="px")
            py = sp.tile([P, J], F32, name="py")
            vx = sp.tile([P, J], F32, name="vx")
            vy = sp.tile([P, J], F32, name="vy")
            pxp = sp.tile([P, J], F32, name="pxp")
            pyp = sp.tile([P, J], F32, name="pyp")
            ix = sp.tile([P, J], F32, name="ix")
            iy = sp.tile([P, J], F32, name="iy")

            # init: pos = z0, vel = (z1 - z0)/dt
            nc.vector.tensor_copy(px, z3[0][:, :, 0])
            nc.vector.tensor_copy(py, z3[0][:, :, 1])
            nc.vector.tensor_sub(vx, z3[1][:, :, 0], z3[0][:, :, 0])
            nc.vector.tensor_sub(vy, z3[1][:, :, 1], z3[0][:, :, 1])
            nc.vector.tensor_scalar_mul(vx, vx, float(np.float32(1.0 / DT)))
            nc.vector.tensor_scalar_mul(vy, vy, float(np.float32(1.0 / DT)))

            stt = nc.vector.scalar_tensor_tensor

            def emit_out(t, write_pos):
                """Allocate the [128, 640] out tile for step t, let write_pos
                fill channels 0/1, fill constant channels, DMA to DRAM."""
                ot = op_.tile([P, 5 * J], F32, name="ot", tag="ot")
                o3 = ot.rearrange("p (j c) -> p j c", c=5)
                write_pos(o3)
                sxv = float(sx_g[t])
                nc.scalar.activation(o3[:, :, 2], dummy, AF.Copy, bias=sxv, scale=0.0)
                nc.scalar.activation(o3[:, :, 3], dummy, AF.Copy, bias=sxv, scale=0.0)
                nc.scalar.activation(o3[:, :, 4], dummy, AF.Copy, bias=0.0, scale=0.0)
                eng = nc.sync if t % 2 == 0 else nc.scalar
                eng.dma_start(
                    y_ap[t].rearrange("(p j) c -> p (j c)", p=P), ot
                )

            # estimation steps: obs index t+1, gains a_g[t], b_g[t]
            for t in range(n_est):
                zx = z3[t + 1][:, :, 0]
                zy = z3[t + 1][:, :, 1]
                av = float(a_g[t])
                bv = float(b_g[t])
                stt(pxp, vx, float(dt), px, OP.mult, OP.add)     # pxp = vx*dt + px
                stt(pyp, vy, float(dt), py, OP.mult, OP.add)
                nc.vector.tensor_sub(ix, zx, pxp)                # ix = z - pxp
                nc.vector.tensor_sub(iy, zy, pyp)
                stt(px, ix, av, pxp, OP.mult, OP.add)            # px = a*ix + pxp
                stt(py, iy, av, pyp, OP.mult, OP.add)
                stt(vx, ix, bv, vx, OP.mult, OP.add)             # vx = b*ix + vx
                stt(vy, iy, bv, vy, OP.mult, OP.add)

                def wp(o3, t=t):
                    nc.scalar.copy(o3[:, :, 0], px)
                    nc.scalar.copy(o3[:, :, 1], py)

                emit_out(t, wp)

            # prediction steps: closed form pos + (k*dt)*vel
            for k in range(1, len_pred + 1):
                t = n_est + k - 1
                kdt = float(np.float32(k) * dt)

                def wp(o3, kdt=kdt):
                    stt(o3[:, :, 0], vx, kdt, px, OP.mult, OP.add)
                    stt(o3[:, :, 1], vy, kdt, py, OP.mult, OP.add)

                emit_out(t, wp)

    nc.compile()
    return nc


def kernel(**inputs):
    from concourse import bass_utils

    x_full = np.ascontiguousarray(np.asarray(inputs["inputs"], dtype=np.float32))
    sigma_a = float(np.asarray(inputs["sigma_a"]))
    sigma_obs = float(np.asarray(inputs["sigma_obs"]))
    sigma_init = float(np.asarray(inputs["sigma_init"]))
    len_pred = int(np.asarray(inputs["len_pred"]))
    assert x_full.shape == (T_OBS, B_FULL, 2), x_full.shape

    key = (sigma_a, sigma_obs, sigma_init, len_pred)
    if key not in _CACHE:
        _CACHE[key] = _build(sigma_a, sigma_obs, sigma_init, len_pred)
    nc = _CACHE[key]

    in_maps = [
        {"x": np.ascontiguousarray(x_full[:, c * B_SHARD : (c + 1) * B_SHARD, :])}
        for c in range(N_CORES)
    ]
    res = bass_utils.run_bass_kernel_spmd(nc, in_maps, core_ids=list(range(N_CORES)))
    outs = [r["y"] for r in res.results]
    return np.concatenate(outs, axis=1)


if __name__ == "__main__":
    import ref_np

    inp = ref_np.setup_inputs_np()
    out = kernel(**inp)
    exp = ref_np.reference_np(
        inp["inputs"], inp["sigma_a"], inp["sigma_obs"], inp["sigma_init"],
        int(inp["len_pred"]))
    err = np.abs(out - exp).max()
    print("max abs err vs ref_np:", err, " rel:", err / np.abs(exp).max())


# revision 4
# speedup vs baseline: 1.0460x; 1.0460x over previous
"""Trainium2 Bass kernel for the batched constant-velocity Kalman filter.

Key structure exploited:
  * The Kalman covariance recursion is data-independent, so the per-step
    gains and output stats (sx, sy, rho) are batch-wide scalars computed on
    host. rho is exactly 0 (x/y decoupled), and sx == sy.
  * Only the state mean is per-trajectory work: a short scalar-gain
    recursion over 9 observation steps, then a closed-form linear
    extrapolation for the prediction steps.
  * Output is [T_est+len_pred, B, 5] = ~102 MB -> the kernel is dominated
    by the output DMA writes; compute (DVE/ACT elementwise) hides under it.

Sharding: pure data parallel over batch, B=131072 -> 16384 per core x 8.

Per-core layout: batch shard as [128 partitions x 128 lanes], b = p*128 + j.
x/y channels stay interleaved: state tiles are [128, 256] = (j, c) pairs, so
each vector op processes both channels at once. Output steps are grouped
G=4 per SBUF tile [128, 4*640] and written with one contiguous-run DMA per
group (2560 B runs per partition per step), alternating the two HWDGE rings.
The estimation recursion writes its position state directly into the output
tiles (strided, f32 two-operand ops are 1x regardless), so no copy ops.
"""

import numpy as np

DT = 0.1
EPS = 0.01
N_CORES = 8
B_FULL = 131072
B_SHARD = B_FULL // N_CORES  # 16384
T_OBS = 10
P = 128                       # SBUF partitions
J = B_SHARD // P              # 128 lanes per partition
G = 4                         # output steps per DMA group


def _scalar_kalman(sigma_a, sigma_obs, sigma_init, n_est, len_pred):
    """Host-side data-independent 2x2 covariance recursion (float64)."""
    sa2 = float(sigma_a) ** 2
    r = float(sigma_obs) ** 2
    F = np.array([[1.0, DT], [0.0, 1.0]])
    Gm = np.array([DT * DT / 2.0, DT])
    Q = sa2 * np.outer(Gm, Gm)
    Pc = (float(sigma_init) ** 2) * np.eye(2)
    a_l, b_l, sx_l = [], [], []
    for _ in range(n_est):
        Pc = F @ Pc @ F.T + Q
        S = Pc[0, 0] + r
        a = Pc[0, 0] / S
        b = Pc[1, 0] / S
        IKH = np.array([[1.0 - a, 0.0], [-b, 1.0]])
        Pc = IKH @ Pc @ IKH.T + r * np.outer([a, b], [a, b])
        a_l.append(a)
        b_l.append(b)
        sx_l.append(np.sqrt(max(Pc[0, 0], EPS * EPS)))
    for _ in range(len_pred):
        Pc = F @ Pc @ F.T + Q
        sx_l.append(np.sqrt(max(Pc[0, 0], EPS * EPS)))
    return np.array(a_l), np.array(b_l), np.array(sx_l)


_CACHE = {}


def _build(sigma_a, sigma_obs, sigma_init, len_pred):
    import concourse.bacc as bacc
    import concourse.mybir as mybir
    import concourse.tile as tile

    AF = mybir.ActivationFunctionType
    OP = mybir.AluOpType
    F32 = mybir.dt.float32

    n_est = T_OBS - 1
    n_out = n_est + len_pred
    a_g, b_g, sx_g = _scalar_kalman(sigma_a, sigma_obs, sigma_init, n_est, len_pred)
    a_g = a_g.astype(np.float32)
    b_g = b_g.astype(np.float32)
    sx_g = sx_g.astype(np.float32)
    dt = float(np.float32(DT))

    # output-step groups: ramp up quickly, then G per group
    groups = []
    t0 = 0
    for sz in [2, 2]:
        groups.append((t0, sz))
        t0 += sz
    while t0 < n_out:
        sz = min(G, n_out - t0)
        groups.append((t0, sz))
        t0 += sz

    nc = bacc.Bacc(
        "TRN2",
        target_bir_lowering=False,
        debug=False,
        enable_asserts=False,
        num_devices=N_CORES,
    )
    x = nc.dram_tensor("x", [T_OBS, B_SHARD, 2], F32, kind="ExternalInput")
    y = nc.dram_tensor("y", [n_out, B_SHARD, 5], F32, kind="ExternalOutput")
    x_ap = x.ap()
    y_ap = y.ap()

    with tile.TileContext(nc) as tc:
        with (
            tc.tile_pool(name="zp", bufs=1) as zp,
            tc.tile_pool(name="sp", bufs=1) as sp,
            tc.tile_pool(name="gp", bufs=3) as gp,
        ):
            # --- input loads: z0,z1 first on the sync ring (fast path),
            # the rest via gpsimd SWDGE so output DMAs are not queued
            # behind them on the HWDGE rings.
            z01 = zp.tile([P, 2 * 2 * J], F32, name="z01")
            nc.sync.dma_start(
                z01.rearrange("p (s f) -> p s f", s=2),
                x_ap[0:2].rearrange("s (p j) c -> p s (j c)", p=P),
            )
            z29 = zp.tile([P, 8 * 2 * J], F32, name="z29")
            nc.gpsimd.dma_start(
                z29.rearrange("p (s f) -> p s f", s=8),
                x_ap[2:T_OBS].rearrange("s (p j) c -> p s (j c)", p=P),
            )

            def zv(s):
                """[128, 256] (j,c)-interleaved view of observation step s."""
                if s < 2:
                    return z01[:, s * 2 * J : (s + 1) * 2 * J]
                return z29[:, (s - 2) * 2 * J : (s - 1) * 2 * J]

            dummy = sp.tile([P, 2 * J], F32, name="dummy")
            nc.vector.memset(dummy, 0.0)

            # persistent state tiles ((j,c) interleaved)
            pxy0 = sp.tile([P, 2 * J], F32, name="pxy0")   # pos before step 0
            pxy9 = sp.tile([P, 2 * J], F32, name="pxy9")   # pos after last est
            vxy = sp.tile([P, 2 * J], F32, name="vxy")
            pp = sp.tile([P, 2 * J], F32, name="pp")       # predicted pos
            ixy = sp.tile([P, 2 * J], F32, name="ixy")     # innovation

            # init: pos = z0, vel = (z1 - z0)/dt
            nc.vector.tensor_copy(pxy0, zv(0))
            nc.vector.tensor_sub(ixy, zv(1), zv(0))
            nc.vector.tensor_scalar_mul(vxy, ixy, float(np.float32(1.0 / DT)))

            stt_v = nc.vector.scalar_tensor_tensor
            stt_g = nc.gpsimd.scalar_tensor_tensor

            # group tiles are allocated lazily below; pos_view[t] is the
            # strided [128, 128, 2] AP of step t's pos channels inside its
            # group tile (written by the recursion, read by step t+1).
            pos_view = {}
            n_slot_init = 0

            for gi, (t0, sz) in enumerate(groups):
                gt = gp.tile([P, G * 5 * J], F32, name="gt", tag="gt")
                g4 = gt.rearrange("p (t j c) -> p t j c", t=G, c=5)
                if n_slot_init < 3:
                    # first occupant of each of the 3 slots zeroes the rho
                    # channel over the full G-step range once; later
                    # occupants inherit the zeros (slot memory is stable).
                    nc.gpsimd.memset(g4[:, :, :, 4], 0.0)
                    n_slot_init += 1
                for ti in range(sz):
                    t = t0 + ti
                    opos = g4[:, ti, :, 0:2]
                    # constant channels sx, sy in one fused ACT op
                    nc.scalar.activation(
                        g4[:, ti, :, 2:4], dummy, AF.Copy,
                        bias=float(sx_g[t]), scale=0.0,
                    )
                    if t < n_est:
                        # estimation step t (obs index t+1)
                        prev = pxy0 if t == 0 else pos_view[t - 1]
                        stt_v(pp, vxy, dt, prev, OP.mult, OP.add)
                        nc.vector.tensor_sub(ixy, zv(t + 1), pp)
                        stt_v(opos, ixy, float(a_g[t]), pp, OP.mult, OP.add)
                        stt_v(vxy, ixy, float(b_g[t]), vxy, OP.mult, OP.add)
                        pos_view[t] = opos
                        if t == n_est - 1:
                            # detach final pos state from the group tile so
                            # prediction steps do not pin this slot
                            nc.vector.tensor_copy(pxy9, opos)
                    else:
                        # prediction step: pos = pxy9 + (k*dt)*vxy
                        k = t - n_est + 1
                        kdt = float(np.float32(k) * np.float32(DT))
                        stt_v(opos, vxy, kdt, pxy9, OP.mult, OP.add)
                # one DMA per group, alternating the two HWDGE rings
                eng = nc.sync if gi % 2 == 0 else nc.scalar
                eng.dma_start(
                    y_ap[t0 : t0 + sz].rearrange("t (p j) c -> p t (j c)", p=P),
                    gt.rearrange("p (t f) -> p t f", t=G)[:, :sz, :],
                )

    nc.compile()
    return nc


def kernel(**inputs):
    from concourse import bass_utils

    x_full = np.ascontiguousarray(np.asarray(inputs["inputs"], dtype=np.float32))
    sigma_a = float(np.asarray(inputs["sigma_a"]))
    sigma_obs = float(np.asarray(inputs["sigma_obs"]))
    sigma_init = float(np.asarray(inputs["sigma_init"]))
    len_pred = int(np.asarray(inputs["len_pred"]))
    assert x_full.shape == (T_OBS, B_FULL, 2), x_full.shape

    key = (sigma_a, sigma_obs, sigma_init, len_pred)
    if key not in _CACHE:
        _CACHE[key] = _build(sigma_a, sigma_obs, sigma_init, len_pred)
    nc = _CACHE[key]

    in_maps = [
        {"x": np.ascontiguousarray(x_full[:, c * B_SHARD : (c + 1) * B_SHARD, :])}
        for c in range(N_CORES)
    ]
    res = bass_utils.run_bass_kernel_spmd(nc, in_maps, core_ids=list(range(N_CORES)))
    outs = [r["y"] for r in res.results]
    return np.concatenate(outs, axis=1)


if __name__ == "__main__":
    import ref_np

    inp = ref_np.setup_inputs_np()
    out = kernel(**inp)
    exp = ref_np.reference_np(
        inp["inputs"], inp["sigma_a"], inp["sigma_obs"], inp["sigma_init"],
        int(inp["len_pred"]))
    err = np.abs(out - exp).max()
    print("max abs err vs ref_np:", err, " rel:", err / np.abs(exp).max())


# revision 6
# speedup vs baseline: 1.0689x; 1.0219x over previous
"""Trainium2 Bass kernel for the batched constant-velocity Kalman filter.

Key structure exploited:
  * The Kalman covariance recursion is data-independent, so the per-step
    gains and output stats (sx, sy, rho) are batch-wide scalars computed on
    host. rho is exactly 0 (x/y decoupled), and sx == sy.
  * Only the state mean is per-trajectory work: a short scalar-gain
    recursion over 9 observation steps, then a closed-form linear
    extrapolation for the prediction steps.
  * Output is [T_est+len_pred, B, 5] = ~102 MB -> the kernel is dominated
    by the output DMA writes; compute (DVE/ACT elementwise) hides under it.

Sharding: pure data parallel over batch, B=131072 -> 16384 per core x 8.

Per-core layout: batch shard as [128 partitions x 128 lanes], b = p*128 + j.
x/y channels stay interleaved: state tiles are [128, 256] = (j, c) pairs, so
each vector op processes both channels at once. Output steps are grouped
G=4 per SBUF tile [128, 4*640] and written with one contiguous-run DMA per
group (2560 B runs per partition per step), alternating the two HWDGE rings.
The estimation recursion writes its position state directly into the output
tiles (strided, f32 two-operand ops are 1x regardless), so no copy ops.
"""

import numpy as np

DT = 0.1
EPS = 0.01
N_CORES = 8
B_FULL = 131072
B_SHARD = B_FULL // N_CORES  # 16384
T_OBS = 10
P = 128                       # SBUF partitions
J = B_SHARD // P              # 128 lanes per partition
G = 4                         # output steps per DMA group


def _scalar_kalman(sigma_a, sigma_obs, sigma_init, n_est, len_pred):
    """Host-side data-independent 2x2 covariance recursion (float64)."""
    sa2 = float(sigma_a) ** 2
    r = float(sigma_obs) ** 2
    F = np.array([[1.0, DT], [0.0, 1.0]])
    Gm = np.array([DT * DT / 2.0, DT])
    Q = sa2 * np.outer(Gm, Gm)
    Pc = (float(sigma_init) ** 2) * np.eye(2)
    a_l, b_l, sx_l = [], [], []
    for _ in range(n_est):
        Pc = F @ Pc @ F.T + Q
        S = Pc[0, 0] + r
        a = Pc[0, 0] / S
        b = Pc[1, 0] / S
        IKH = np.array([[1.0 - a, 0.0], [-b, 1.0]])
        Pc = IKH @ Pc @ IKH.T + r * np.outer([a, b], [a, b])
        a_l.append(a)
        b_l.append(b)
        sx_l.append(np.sqrt(max(Pc[0, 0], EPS * EPS)))
    for _ in range(len_pred):
        Pc = F @ Pc @ F.T + Q
        sx_l.append(np.sqrt(max(Pc[0, 0], EPS * EPS)))
    return np.array(a_l), np.array(b_l), np.array(sx_l)


_CACHE = {}


def _build(sigma_a, sigma_obs, sigma_init, len_pred):
    import concourse.bacc as bacc
    import concourse.mybir as mybir
    import concourse.tile as tile

    AF = mybir.ActivationFunctionType
    OP = mybir.AluOpType
    F32 = mybir.dt.float32

    n_est = T_OBS - 1
    n_out = n_est + len_pred
    a_g, b_g, sx_g = _scalar_kalman(sigma_a, sigma_obs, sigma_init, n_est, len_pred)
    a_g = a_g.astype(np.float32)
    b_g = b_g.astype(np.float32)
    sx_g = sx_g.astype(np.float32)
    dt = float(np.float32(DT))

    # output-step groups: ramp up quickly, then G per group
    groups = []
    t0 = 0
    for sz in [2, 2]:
        groups.append((t0, sz))
        t0 += sz
    while t0 < n_out:
        sz = min(G, n_out - t0)
        groups.append((t0, sz))
        t0 += sz

    nc = bacc.Bacc(
        "TRN2",
        target_bir_lowering=False,
        debug=False,
        enable_asserts=False,
        num_devices=N_CORES,
    )
    x = nc.dram_tensor("x", [T_OBS, B_SHARD, 2], F32, kind="ExternalInput")
    y = nc.dram_tensor("y", [n_out, B_SHARD, 5], F32, kind="ExternalOutput")
    x_ap = x.ap()
    y_ap = y.ap()

    with tile.TileContext(nc) as tc:
        with (
            tc.tile_pool(name="zp", bufs=1) as zp,
            tc.tile_pool(name="sp", bufs=1) as sp,
            tc.tile_pool(name="gp", bufs=3) as gp,
        ):
            # --- input loads: z0,z1 first on the sync ring (fast path),
            # the rest via gpsimd SWDGE so output DMAs are not queued
            # behind them on the HWDGE rings.
            z01 = zp.tile([P, 2 * 2 * J], F32, name="z01")
            nc.sync.dma_start(
                z01.rearrange("p (s f) -> p s f", s=2),
                x_ap[0:2].rearrange("s (p j) c -> p s (j c)", p=P),
            )
            z29 = zp.tile([P, 8 * 2 * J], F32, name="z29")
            nc.gpsimd.dma_start(
                z29.rearrange("p (s f) -> p s f", s=8),
                x_ap[2:T_OBS].rearrange("s (p j) c -> p s (j c)", p=P),
            )

            def zv(s):
                """[128, 256] (j,c)-interleaved view of observation step s."""
                if s < 2:
                    return z01[:, s * 2 * J : (s + 1) * 2 * J]
                return z29[:, (s - 2) * 2 * J : (s - 1) * 2 * J]

            dummy = sp.tile([P, 2 * J], F32, name="dummy")
            nc.vector.memset(dummy, 0.0)

            # persistent state tiles ((j,c) interleaved)
            pxy0 = sp.tile([P, 2 * J], F32, name="pxy0")   # pos before step 0
            pxy9 = sp.tile([P, 2 * J], F32, name="pxy9")   # pos after last est
            vxy = sp.tile([P, 2 * J], F32, name="vxy")
            pp = sp.tile([P, 2 * J], F32, name="pp")       # predicted pos
            ixy = sp.tile([P, 2 * J], F32, name="ixy")     # innovation

            # init: pos = z0, vel = (z1 - z0)/dt
            nc.vector.tensor_copy(pxy0, zv(0))
            nc.vector.tensor_sub(ixy, zv(1), zv(0))
            nc.vector.tensor_scalar_mul(vxy, ixy, float(np.float32(1.0 / DT)))

            stt_v = nc.vector.scalar_tensor_tensor
            stt_g = nc.gpsimd.scalar_tensor_tensor

            # group tiles are allocated lazily below; pos_view[t] is the
            # strided [128, 128, 2] AP of step t's pos channels inside its
            # group tile (written by the recursion, read by step t+1).
            pos_view = {}
            n_slot_init = 0

            for gi, (t0, sz) in enumerate(groups):
                gt = gp.tile([P, G * 5 * J], F32, name="gt", tag="gt")
                g4 = gt.rearrange("p (t j c) -> p t j c", t=G, c=5)
                if n_slot_init < 3:
                    # first occupant of each of the 3 slots zeroes the rho
                    # channel over the full G-step range once; later
                    # occupants inherit the zeros (slot memory is stable).
                    nc.vector.memset(g4[:, :, :, 4], 0.0)
                    n_slot_init += 1
                for ti in range(sz):
                    t = t0 + ti
                    opos = g4[:, ti, :, 0:2]
                    # constant channels sx, sy in one fused ACT op
                    nc.scalar.activation(
                        g4[:, ti, :, 2:4], dummy, AF.Copy,
                        bias=float(sx_g[t]), scale=0.0,
                    )
                    if t < n_est:
                        # estimation step t (obs index t+1)
                        prev = pxy0 if t == 0 else pos_view[t - 1]
                        stt_v(pp, vxy, dt, prev, OP.mult, OP.add)
                        nc.vector.tensor_sub(ixy, zv(t + 1), pp)
                        stt_v(opos, ixy, float(a_g[t]), pp, OP.mult, OP.add)
                        stt_v(vxy, ixy, float(b_g[t]), vxy, OP.mult, OP.add)
                        pos_view[t] = opos
                        if t == n_est - 1:
                            # detach final pos state from the group tile so
                            # prediction steps do not pin this slot
                            nc.vector.tensor_copy(pxy9, opos)
                    else:
                        # prediction step: pos = pxy9 + (k*dt)*vxy
                        k = t - n_est + 1
                        kdt = float(np.float32(k) * np.float32(DT))
                        stt_v(opos, vxy, kdt, pxy9, OP.mult, OP.add)
                # one DMA per group, alternating the two HWDGE rings
                eng = nc.sync if gi % 2 == 0 else nc.scalar
                eng.dma_start(
                    y_ap[t0 : t0 + sz].rearrange("t (p j) c -> p t (j c)", p=P),
                    gt.rearrange("p (t f) -> p t f", t=G)[:, :sz, :],
                )

    nc.compile()
    return nc


def kernel(**inputs):
    from concourse import bass_utils

    x_full = np.ascontiguousarray(np.asarray(inputs["inputs"], dtype=np.float32))
    sigma_a = float(np.asarray(inputs["sigma_a"]))
    sigma_obs = float(np.asarray(inputs["sigma_obs"]))
    sigma_init = float(np.asarray(inputs["sigma_init"]))
    len_pred = int(np.asarray(inputs["len_pred"]))
    assert x_full.shape == (T_OBS, B_FULL, 2), x_full.shape

    key = (sigma_a, sigma_obs, sigma_init, len_pred)
    if key not in _CACHE:
        _CACHE[key] = _build(sigma_a, sigma_obs, sigma_init, len_pred)
    nc = _CACHE[key]

    in_maps = [
        {"x": np.ascontiguousarray(x_full[:, c * B_SHARD : (c + 1) * B_SHARD, :])}
        for c in range(N_CORES)
    ]
    res = bass_utils.run_bass_kernel_spmd(nc, in_maps, core_ids=list(range(N_CORES)))
    outs = [r["y"] for r in res.results]
    return np.concatenate(outs, axis=1)


if __name__ == "__main__":
    import ref_np

    inp = ref_np.setup_inputs_np()
    out = kernel(**inp)
    exp = ref_np.reference_np(
        inp["inputs"], inp["sigma_a"], inp["sigma_obs"], inp["sigma_init"],
        int(inp["len_pred"]))
    err = np.abs(out - exp).max()
    print("max abs err vs ref_np:", err, " rel:", err / np.abs(exp).max())
